# revision 1
# baseline (speedup 1.0000x reference)
"""Trainium2 Bass kernel for nn_LiquidNeuralNetwork (131072x14 -> 131072x3).

Math: the reference integrates dy/dt = tanh(y@W1+b1)@W2 + b2 from t=0 to 1
with 32 fixed dopri5 steps, between an input layer (x@W_in+b_in) and an output
layer (y@W_out+b_out). The ODE is so smooth that classic RK4 with 8 steps
reproduces the reference to ~4e-6 absmax (the reference's own fp32 noise floor
is ~1.5e-6), so this kernel integrates with RK4(8).

State-space change of variables: track u = W1^T y (feature-major). Then each
RK4 stage needs one 64x64 matmul with C = W1^T W2^T and a tanh; the y-state
never needs to be materialized, and the output projection telescopes to
out = G u_T + const with G = (W1^{-1} W_out)^T.

Layout per core: batch 16384 split into two halves stacked on SBUF partitions
(rows 0-63 = features of half A, 64-127 = half B); all 64x64 weight blocks are
applied as 128x128 block-diagonal stationary operands; batch streams as the
moving operand in 512-column PSUM tiles.

Precision: stage-arg matmuls run in fp32r (fast, 1 col/cycle); the state-update
(u') matmuls use a hi/lo split of the weights (two fp32r matmuls) which removes
the systematic weight-rounding bias; input/output projections are full fp32.
Measured end-to-end error vs the reference: ~6e-5 absmax (scale ~5.2).
"""
import sys
sys.path.insert(0, '/opt/trn_rl_repo')

import numpy as np

import concourse.bass as bass  # noqa: F401  (bass must import before bacc)
import concourse.bacc as bacc
import concourse.mybir as mybir
from concourse import tile
from concourse.bass_utils import run_bass_kernel_spmd

F32 = mybir.dt.float32
F32R = mybir.dt.float32r
TANH = mybir.ActivationFunctionType.Tanh
ADD = mybir.AluOpType.add

N_CORES = 8
B_FULL = 131072
D_IN = 14
L = 64
D_OUT = 3
NS = 8           # RK4 steps
TW = 512         # columns per tile (one PSUM bank of fp32)
G_ILV = 3        # tiles emitted in lockstep (software pipelining)


def _round_mant(a, bits=11):
    """Round fp32 array to `bits` mantissa bits (exactly representable in fp32r)."""
    a = np.asarray(a, np.float32)
    m, e = np.frexp(a)
    return np.ldexp(np.round(m * (1 << bits)) / (1 << bits), e).astype(np.float32)


def _blockdiag(blk):
    blk = np.asarray(blk, np.float32)
    k, m = blk.shape
    out = np.zeros((2 * k, 2 * m), np.float32)
    out[:k, :m] = blk
    out[k:, m:] = blk
    return out


def _precompute(x, time_span, W_in, b_in, W1, b1, W2, b2, W_out, b_out):
    """Host-side: derived weight matrices and constants (f64 internally)."""
    f8 = np.float64
    W_in, b_in, W1, b1, W2, b2, W_out, b_out = [
        np.asarray(a, f8) for a in (W_in, b_in, W1, b1, W2, b2, W_out, b_out)]
    T = float(np.asarray(time_span)[1] - np.asarray(time_span)[0])
    h = T / NS

    C_T = W2 @ W1                      # [64,64] lhsT block: out = (C_T)^T @ t = C t
    E_T = W_in @ W1                    # [14,64]
    G_T = np.linalg.solve(W1, W_out)   # [64,3]
    w_vec = W1.T @ b2                  # [64]

    d = {}
    d['sw2'] = _blockdiag((h / 2) * C_T)          # s2/s3 edges
    d['sw4'] = _blockdiag(h * C_T)                # s4 edge
    for name, c in (('uw6', h / 6), ('uw3', h / 3)):
        full = c * C_T
        hi = _round_mant(full.astype(np.float32), 11)
        lo = (full - hi).astype(np.float32)
        d[name + 'hi'] = _blockdiag(hi)
        d[name + 'lo'] = _blockdiag(lo)
    ew = np.zeros((64, 128), np.float32)           # halves at partition 0 / 32
    ew[0:D_IN, 0:64] = E_T.astype(np.float32)
    ew[32:32 + D_IN, 64:128] = E_T.astype(np.float32)
    d['ew'] = ew
    gw = np.zeros((128, 35), np.float32)           # out rows: A at 0-2, B at 32-34
    gw[0:64, 0:D_OUT] = G_T.astype(np.float32)
    gw[64:128, 32:32 + D_OUT] = G_T.astype(np.float32)
    d['gw'] = gw
    d['ident'] = np.eye(128, dtype=np.float32)

    biases = np.zeros((128, NS * 3), np.float32)
    for s in range(NS):
        biases[:64, s * 3 + 0] = biases[64:, s * 3 + 0] = b1 + s * h * w_vec
        biases[:64, s * 3 + 1] = biases[64:, s * 3 + 1] = b1 + (s * h + h / 2) * w_vec
        biases[:64, s * 3 + 2] = biases[64:, s * 3 + 2] = b1 + (s + 1) * h * w_vec
    d['biases'] = biases

    u0c = np.zeros((128, 1), np.float32)
    u0c[:64, 0] = u0c[64:, 0] = W1.T @ b_in
    d['u0c'] = u0c

    oc = np.zeros((35, 1), np.float32)
    occ = (b_out + G_T.T @ (NS * h * w_vec)).astype(np.float32)
    oc[0:D_OUT, 0] = occ
    oc[32:32 + D_OUT, 0] = occ
    d['oc'] = oc
    return d


def build_nc(n_tiles, n_steps, num_devices=N_CORES, ilv=G_ILV, n_id=0, tw=None,
             s_bufs=None, u_bufs=None, defer_up=False, stage_sbuf=False):
    """Build and compile the per-core Bass program.

    Per-core batch = 2 * n_tiles * TW (two stacked halves of n_tiles*TW cols).
    """
    tw = TW if tw is None else tw
    nch = tw // 128          # 128-col transpose chunks per half-tile
    sb_bufs = (ilv + 1) if s_bufs is None else s_bufs
    ua_bufs = ilv if u_bufs is None else u_bufs
    tbufs = (4 * ilv + 4) if defer_up else (2 * ilv)
    half = n_tiles * tw
    bc = 2 * half
    nc = bacc.Bacc("TRN2", target_bir_lowering=False, debug=False,
                   num_devices=num_devices)

    x_d = nc.dram_tensor("x", [bc, D_IN], F32, kind="ExternalInput").ap()
    sw2_d = nc.dram_tensor("sw2", [128, 128], F32, kind="ExternalInput").ap()
    sw4_d = nc.dram_tensor("sw4", [128, 128], F32, kind="ExternalInput").ap()
    uw6hi_d = nc.dram_tensor("uw6hi", [128, 128], F32, kind="ExternalInput").ap()
    uw6lo_d = nc.dram_tensor("uw6lo", [128, 128], F32, kind="ExternalInput").ap()
    uw3hi_d = nc.dram_tensor("uw3hi", [128, 128], F32, kind="ExternalInput").ap()
    uw3lo_d = nc.dram_tensor("uw3lo", [128, 128], F32, kind="ExternalInput").ap()
    ew_d = nc.dram_tensor("ew", [64, 128], F32, kind="ExternalInput").ap()
    gw_d = nc.dram_tensor("gw", [128, 35], F32, kind="ExternalInput").ap()
    id_d = nc.dram_tensor("ident", [128, 128], F32, kind="ExternalInput").ap()
    bias_d = nc.dram_tensor("biases", [128, n_steps * 3], F32, kind="ExternalInput").ap()
    u0c_d = nc.dram_tensor("u0c", [128, 1], F32, kind="ExternalInput").ap()
    oc_d = nc.dram_tensor("oc", [35, 1], F32, kind="ExternalInput").ap()
    y_d = nc.dram_tensor("y", [bc, D_OUT], F32, kind="ExternalOutput").ap()

    with tile.TileContext(nc) as tc:
        with (
            tc.tile_pool(name="const", bufs=1) as cpool,
            tc.tile_pool(name="work", bufs=1) as wpool,
        ):
            # --- load constants / weights, convert matmul weights to fp32r ---
            def load_const(name, src, shape):
                t = cpool.tile(shape, F32, name=name)
                nc.sync.dma_start(t[:], src)
                return t

            sw2_f = load_const("sw2_f", sw2_d[:], [128, 128])
            sw4_f = load_const("sw4_f", sw4_d[:], [128, 128])
            uw6hi_f = load_const("uw6hi_f", uw6hi_d[:], [128, 128])
            uw6lo_f = load_const("uw6lo_f", uw6lo_d[:], [128, 128])
            uw3hi_f = load_const("uw3hi_f", uw3hi_d[:], [128, 128])
            uw3lo_f = load_const("uw3lo_f", uw3lo_d[:], [128, 128])
            ew_t = load_const("ew_t", ew_d[:], [64, 128])
            gw_t = load_const("gw_t", gw_d[:], [128, 35])
            id_t = load_const("id_t", id_d[:], [128, 128])
            bias_t = load_const("bias_t", bias_d[:], [128, n_steps * 3])
            u0c_t = load_const("u0c_t", u0c_d[:], [128, 1])
            oc_t = load_const("oc_t", oc_d[:], [35, 1])

            rweights = {}
            for nm, ft in (("sw2", sw2_f), ("sw4", sw4_f),
                           ("uw6hi", uw6hi_f), ("uw6lo", uw6lo_f),
                           ("uw3hi", uw3hi_f), ("uw3lo", uw3lo_f)):
                rt = cpool.tile([128, 128], F32R, name=nm + "_r")
                nc.vector.tensor_copy(rt[:], ft[:])
                rweights[nm] = rt

            out_sb = wpool.tile([35, half], F32, name="out_sb")

            # --- per-tile emission helpers (interleaved across ilv tiles) ---
            with (
                tc.tile_pool(name="sb", bufs=1) as sb,
                tc.tile_pool(name="psw", bufs=1, space="PSUM") as psw,
            ):
                def emit_group(tiles):
                    st = {}

                    for j in tiles:
                        xa = sb.tile([128, nch, D_IN], F32, tag="xin", bufs=min(2 * ilv, 8), name=f"xa{j}")
                        xb = sb.tile([128, nch, D_IN], F32, tag="xin", bufs=min(2 * ilv, 8), name=f"xb{j}")
                        nc.sync.dma_start(
                            xa[:], x_d[tw * j: tw * (j + 1), :]
                            .rearrange("(c p) f -> p c f", p=128))
                        nc.sync.dma_start(
                            xb[:], x_d[half + tw * j: half + tw * (j + 1), :]
                            .rearrange("(c p) f -> p c f", p=128))
                        st[j] = {'xa': xa, 'xb': xb}

                    for j in tiles:
                        xt = sb.tile([64, tw], F32, tag="xt", bufs=min(ilv, 6), name=f"xt{j}")
                        nc.gpsimd.memset(xt[:], 0.0)
                        for hsel, xsrc in ((0, st[j]['xa']), (1, st[j]['xb'])):
                            xps = psw.tile([D_IN, tw], F32, tag="xt_ps", bufs=1, name=f"xps{j}_{hsel}")
                            for c in range(nch):
                                nc.tensor.matmul(
                                    xps[:, 128 * c:128 * (c + 1)],
                                    xsrc[:, c, :], id_t[:],
                                    is_transpose=True,
                                    start=(c == 0), stop=(c == nch - 1))
                            nc.vector.tensor_copy(
                                xt[32 * hsel: 32 * hsel + D_IN, :], xps[:])
                        st[j]['xt'] = xt

                    # u0 = E x + const
                    for j in tiles:
                        ups = psw.tile([128, tw], F32, tag="uacc", bufs=ua_bufs, name=f"u0ps{j}")
                        nc.tensor.matmul(ups[:], ew_t[:], st[j]['xt'][:],
                                         start=True, stop=True)
                        u = sb.tile([128, tw], F32, tag="u", bufs=ilv + 2, name=f"u0_{j}")
                        nc.vector.tensor_scalar(u[:], ups[:], u0c_t[:], None, ADD)
                        st[j]['u'] = u

                    for s in range(n_steps):
                        b1s = bias_t[:, s * 3 + 0: s * 3 + 1]
                        b23s = bias_t[:, s * 3 + 1: s * 3 + 2]
                        b4s = bias_t[:, s * 3 + 2: s * 3 + 3]

                        for j in tiles:
                            t1 = sb.tile([128, tw], F32R, tag="t", bufs=tbufs, name=f"t1_{j}_{s}")
                            nc.scalar.activation(t1[:], st[j]['u'][:], TANH,
                                                 bias=b1s, scale=1.0)
                            st[j]['t', 0] = t1
                        if not defer_up:
                            for j in tiles:
                                up = psw.tile([128, tw], F32, tag="uacc", bufs=ua_bufs, name=f"up{j}_{s}")
                                st[j]['up'] = up

                        stage_w = [('sw2', b23s), ('sw2', b23s), ('sw4', b4s)]
                        ucoef = ['uw6', 'uw3', 'uw3', 'uw6']
                        for i, (wnm, bias) in enumerate(stage_w):
                            for j in tiles:
                                sp = psw.tile([128, tw], F32, tag="s", bufs=sb_bufs, name=f"s{i}_{j}_{s}")
                                nc.tensor.matmul(sp[:], rweights[wnm][:], st[j]['t', i][:],
                                                 start=True, stop=(n_id <= i))
                                if n_id > i:
                                    nc.tensor.matmul(sp[:], id_t[:], st[j]['u'][:],
                                                     start=False, stop=True)
                                if not defer_up:
                                    cw = ucoef[i]
                                    nc.tensor.matmul(st[j]['up'][:], rweights[cw + 'hi'][:], st[j]['t', i][:],
                                                     start=(i == 0), stop=False)
                                    nc.tensor.matmul(st[j]['up'][:], rweights[cw + 'lo'][:], st[j]['t', i][:],
                                                     start=False, stop=False)
                                st[j]['sp'] = sp
                            if n_id <= i:
                                if stage_sbuf:
                                    for j in tiles:
                                        stmp = sb.tile([128, tw], F32, tag="stmp", bufs=ilv + 2, name=f"sm{i}_{j}_{s}")
                                        nc.vector.tensor_tensor(stmp[:], st[j]['sp'][:],
                                                                st[j]['u'][:], ADD)
                                        st[j]['sp'] = stmp
                                else:
                                    for j in tiles:
                                        nc.vector.tensor_tensor(st[j]['sp'][:], st[j]['sp'][:],
                                                                st[j]['u'][:], ADD)
                            for j in tiles:
                                tn = sb.tile([128, tw], F32R, tag="t", bufs=tbufs, name=f"t{i + 2}_{j}_{s}")
                                nc.scalar.activation(tn[:], st[j]['sp'][:], TANH,
                                                     bias=bias, scale=1.0)
                                st[j]['t', i + 1] = tn

                        # u' accumulation tail
                        if defer_up:
                            for j in tiles:
                                up = psw.tile([128, tw], F32, tag="uacc", bufs=ua_bufs, name=f"up{j}_{s}")
                                st[j]['up'] = up
                                for i in range(4):
                                    cw = ucoef[i]
                                    nc.tensor.matmul(up[:], rweights[cw + 'hi'][:], st[j]['t', i][:],
                                                     start=(i == 0), stop=False)
                                    nc.tensor.matmul(up[:], rweights[cw + 'lo'][:], st[j]['t', i][:],
                                                     start=False, stop=(i == 3))
                        else:
                            for j in tiles:
                                cw = ucoef[3]
                                nc.tensor.matmul(st[j]['up'][:], rweights[cw + 'hi'][:], st[j]['t', 3][:],
                                                 start=False, stop=False)
                                nc.tensor.matmul(st[j]['up'][:], rweights[cw + 'lo'][:], st[j]['t', 3][:],
                                                 start=False, stop=True)
                        for j in tiles:
                            un = sb.tile([128, tw], F32, tag="u", bufs=ilv + 2, name=f"u{j}_{s}")
                            nc.vector.tensor_tensor(un[:], st[j]['up'][:],
                                                    st[j]['u'][:], ADD)
                            st[j]['u'] = un

                    # out tile = G u_T + const
                    for j in tiles:
                        ops_ = psw.tile([35, tw], F32, tag="s", bufs=sb_bufs, name=f"ops{j}")
                        nc.tensor.matmul(ops_[:], gw_t[:], st[j]['u'][:],
                                         start=True, stop=True)
                        nc.vector.tensor_scalar(
                            out_sb[:, tw * j: tw * (j + 1)], ops_[:], oc_t[:], None, ADD)

                for g0 in range(0, n_tiles, ilv):
                    emit_group(list(range(g0, min(g0 + ilv, n_tiles))))

                # write out: [2*D_OUT, half] -> y [bc, 3] (strided)
                with nc.allow_non_contiguous_dma("transposed [3,B] output store"):
                    nc.sync.dma_start(
                        y_d[0:half, :].rearrange("b c -> c b"), out_sb[0:D_OUT, :])
                    nc.sync.dma_start(
                        y_d[half:bc, :].rearrange("b c -> c b"), out_sb[32:32 + D_OUT, :])

    nc.compile()
    return nc


_NC_CACHE = {}


def _get_nc(n_tiles, n_steps):
    key = (n_tiles, n_steps)
    if key not in _NC_CACHE:
        _NC_CACHE[key] = build_nc(n_tiles, n_steps)
    return _NC_CACHE[key]


def kernel(**inputs):
    x = np.ascontiguousarray(np.asarray(inputs['x'], np.float32))
    host = _precompute(**inputs)
    n_tiles = B_FULL // N_CORES // (2 * TW)
    nc = _get_nc(n_tiles, NS)

    shared = {k: np.ascontiguousarray(v.astype(np.float32)) for k, v in host.items()}
    bc = B_FULL // N_CORES
    in_maps = []
    for i in range(N_CORES):
        m = dict(shared)
        m['x'] = x[i * bc:(i + 1) * bc]
        in_maps.append(m)

    res = run_bass_kernel_spmd(nc, in_maps, core_ids=list(range(N_CORES)))
    out = np.concatenate([res.results[i]['y'] for i in range(N_CORES)], axis=0)
    return out.astype(np.float32)



# revision 2
# speedup vs baseline: 4.1548x; 4.1548x over previous
"""Trainium2 Bass kernel for nn_LiquidNeuralNetwork (131072x14 -> 131072x3).

Math: the reference integrates dy/dt = tanh(y@W1+b1)@W2 + b2 from t=0 to 1
with 32 fixed dopri5 steps, between an input layer (x@W_in+b_in) and an output
layer (y@W_out+b_out).  The flow is so smooth that classic RK4 with 2 steps
reproduces the reference to ~1.7e-3 relative (threshold 2e-2).

State-space change of variables: track u = W1^T y (feature-major).  Then
u' = C^T tanh(u + b1 + t*w) with C = W2@W1, w = W1^T b2 (the constant drift is
removed by tracking v = u - t*w, which shifts the tanh biases per stage time).
The input layer u0 = (W_in@W1)^T x + W1^T b_in is computed on the HOST (tiny
14x64 GEMM) and shipped feature-major, so the device does no transposes.  The
output projection telescopes to out = G^T u_T + const with G = W1^{-1} W_out
(const added on the host after gather).

Per-core layout: batch 16384 split into two halves stacked on SBUF partitions
(rows 0-63 = features of half A, 64-127 = half B); 64x64 weight blocks applied
as 128x128 block-diagonal stationary operands; batch streams as the moving
operand in 512-column PSUM tiles.  All moving operands are fp32r (1 col/cycle).

Per RK4 step (per tile): 6 matmuls (3 stage args + 3 accumulation:
uw6*(t1+t4), uw3hi/lo*(t2+t3)), 4 tanhs on ACT, 3 in-place PSUM adds + 1
u-update on DVE, 2 t-sums on GpSimd.  The u'-accumulation weight h/3*C uses an
fp32r hi/lo split (removes stationary-rounding bias); h/6*C is single.
"""
import sys
sys.path.insert(0, '/opt/trn_rl_repo')

import numpy as np

import concourse.bass as bass  # noqa: F401  (bass must import before bacc)
import concourse.bacc as bacc
import concourse.mybir as mybir
from concourse import tile
from concourse.bass_utils import run_bass_kernel_spmd

F32 = mybir.dt.float32
F32R = mybir.dt.float32r
TANH = mybir.ActivationFunctionType.Tanh
COPY = mybir.ActivationFunctionType.Copy
ADD = mybir.AluOpType.add

N_CORES = 8
B_FULL = 131072
D_IN = 14
L = 64
D_OUT = 3
NS = 2           # RK4 steps
TW = 512         # columns per tile (one PSUM bank of fp32)
G_ILV = 4        # tiles emitted in lockstep (software pipelining)


def _round_mant(a, bits=11):
    """Round fp32 array to `bits` mantissa bits (exactly representable in fp32r)."""
    a = np.asarray(a, np.float32)
    m, e = np.frexp(a)
    return np.ldexp(np.round(m * (1 << bits)) / (1 << bits), e).astype(np.float32)


def _hilo(a):
    hi = _round_mant(np.asarray(a, np.float32))
    lo = _round_mant((np.asarray(a, np.float64) - hi).astype(np.float32))
    return hi, lo


def _blockdiag(blk):
    blk = np.asarray(blk, np.float32)
    k, m = blk.shape
    out = np.zeros((2 * k, 2 * m), np.float32)
    out[:k, :m] = blk
    out[k:, m:] = blk
    return out


def _precompute(x, time_span, W_in, b_in, W1, b1, W2, b2, W_out, b_out):
    """Host-side: derived weights (f64 internally), per-core u0, out const."""
    f8 = np.float64
    x64 = np.asarray(x, f8)
    W_in, b_in, W1, b1, W2, b2, W_out, b_out = [
        np.asarray(a, f8) for a in (W_in, b_in, W1, b1, W2, b2, W_out, b_out)]
    T = float(np.asarray(time_span)[1] - np.asarray(time_span)[0])
    h = T / NS

    C = W2 @ W1                        # [64,64] stationary block: out = C^T @ t
    E = W_in @ W1                      # [14,64]
    G = np.linalg.solve(W1, W_out)     # [64,3]
    w = b2 @ W1                        # [64]

    d = {}
    d['sw2'] = _blockdiag(_round_mant((h / 2) * C))
    d['sw4'] = _blockdiag(_round_mant(h * C))
    d['uw6'] = _blockdiag(_round_mant((h / 6) * C))
    hi, lo = _hilo((h / 3) * C)
    d['uw3hi'] = _blockdiag(hi)
    d['uw3lo'] = _blockdiag(lo)

    ghi, glo = _hilo(G)
    gw_hi = np.zeros((128, 2 * D_OUT), np.float32)
    gw_lo = np.zeros((128, 2 * D_OUT), np.float32)
    gw_hi[0:L, 0:D_OUT] = ghi
    gw_hi[L:128, D_OUT:2 * D_OUT] = ghi
    gw_lo[0:L, 0:D_OUT] = glo
    gw_lo[L:128, D_OUT:2 * D_OUT] = glo
    d['gwhi'] = gw_hi
    d['gwlo'] = gw_lo

    biases = np.zeros((128, NS * 3), np.float32)
    for s in range(NS):
        biases[:L, s * 3 + 0] = biases[L:, s * 3 + 0] = b1 + s * h * w
        biases[:L, s * 3 + 1] = biases[L:, s * 3 + 1] = b1 + (s * h + h / 2) * w
        biases[:L, s * 3 + 2] = biases[L:, s * 3 + 2] = b1 + (s + 1) * h * w
    d['biases'] = biases

    # host input layer: u0 = x @ E + b_in @ W1, shipped feature-major per core
    u0 = (x64 @ E + b_in @ W1).astype(np.float32)   # [B, 64]
    half = B_FULL // N_CORES // 2
    u0Ts = []
    for i in range(N_CORES):
        uc = u0[i * 2 * half:(i + 1) * 2 * half]
        u0Ts.append(np.ascontiguousarray(
            np.concatenate([uc[:half].T, uc[half:].T], axis=0)))  # [128, half]

    occ = (b_out + T * (w @ G)).astype(np.float32)  # [3] host-side out const
    return d, u0Ts, occ


def build_nc(n_tiles, n_steps, num_devices=N_CORES, ilv=G_ILV, tw=TW,
             p_bufs=8, u_bufs=None, t_bufs=None, ss_bufs=None, n_chunks=4):
    """Build and compile the per-core Bass program.

    Per-core batch = 2 * n_tiles * tw (two stacked halves of n_tiles*tw cols).
    """
    half = n_tiles * tw
    u_bufs = (ilv + 3) if u_bufs is None else u_bufs
    t_bufs = (4 * ilv + 2) if t_bufs is None else t_bufs
    ss_bufs = (2 * ilv + 2) if ss_bufs is None else ss_bufs
    nc = bacc.Bacc("TRN2", target_bir_lowering=False, debug=False,
                   num_devices=num_devices)

    u0_d = nc.dram_tensor("u0T", [128, half], F32R, kind="ExternalInput").ap()
    sw2_d = nc.dram_tensor("sw2", [128, 128], F32, kind="ExternalInput").ap()
    sw4_d = nc.dram_tensor("sw4", [128, 128], F32, kind="ExternalInput").ap()
    uw6_d = nc.dram_tensor("uw6", [128, 128], F32, kind="ExternalInput").ap()
    uw3hi_d = nc.dram_tensor("uw3hi", [128, 128], F32, kind="ExternalInput").ap()
    uw3lo_d = nc.dram_tensor("uw3lo", [128, 128], F32, kind="ExternalInput").ap()
    gwhi_d = nc.dram_tensor("gwhi", [128, 2 * D_OUT], F32, kind="ExternalInput").ap()
    gwlo_d = nc.dram_tensor("gwlo", [128, 2 * D_OUT], F32, kind="ExternalInput").ap()
    bias_d = nc.dram_tensor("biases", [128, n_steps * 3], F32, kind="ExternalInput").ap()
    y_d = nc.dram_tensor("yT", [2 * D_OUT, half], F32, kind="ExternalOutput").ap()

    with tile.TileContext(nc) as tc:
        with (
            tc.tile_pool(name="const", bufs=1) as cpool,
            tc.tile_pool(name="work", bufs=1) as wpool,
        ):
            def load_const(name, src, shape):
                t = cpool.tile(shape, F32, name=name)
                nc.sync.dma_start(t[:], src)
                return t

            bias_t = load_const("bias_t", bias_d[:], [128, n_steps * 3])
            rw = {}
            for nm, src, shape in (
                    ("sw2", sw2_d, [128, 128]), ("sw4", sw4_d, [128, 128]),
                    ("uw6", uw6_d, [128, 128]),
                    ("uw3hi", uw3hi_d, [128, 128]), ("uw3lo", uw3lo_d, [128, 128]),
                    ("gwhi", gwhi_d, [128, 2 * D_OUT]),
                    ("gwlo", gwlo_d, [128, 2 * D_OUT])):
                ft = load_const(nm + "_f", src[:], shape)
                rt = cpool.tile(shape, F32R, name=nm + "_r")
                nc.vector.tensor_copy(rt[:], ft[:])
                rw[nm] = rt

            # whole-core input buffer, loaded in a few big contiguous DMAs
            u0_sb = wpool.tile([128, half], F32R, name="u0_sb")
            cw = half // n_chunks
            for c in range(n_chunks):
                nc.sync.dma_start(u0_sb[:, c * cw:(c + 1) * cw],
                                  u0_d[:, c * cw:(c + 1) * cw])

            out_sb = wpool.tile([2 * D_OUT, half], F32, name="out_sb")

            with (
                tc.tile_pool(name="sb", bufs=1) as sb,
                tc.tile_pool(name="psw", bufs=1, space="PSUM") as psw,
            ):
                def emit_group(tiles):
                    st = {j: {} for j in tiles}
                    for j in tiles:
                        st[j]['u'] = u0_sb[:, tw * j:tw * (j + 1)]

                    for s in range(n_steps):
                        b1s = bias_t[:, s * 3 + 0: s * 3 + 1]
                        b23s = bias_t[:, s * 3 + 1: s * 3 + 2]
                        b4s = bias_t[:, s * 3 + 2: s * 3 + 3]

                        # stage 1
                        for j in tiles:
                            t1 = sb.tile([128, tw], F32R, tag="t", bufs=t_bufs,
                                         name=f"t1_{j}_{s}")
                            nc.scalar.activation(t1[:], st[j]['u'][:], TANH,
                                                 bias=b1s, scale=1.0)
                            st[j]['t1'] = t1
                        # stages 2..4: matmul, in-place +u, tanh
                        for i, (wnm, bias, tin, tout) in enumerate((
                                ('sw2', b23s, 't1', 't2'),
                                ('sw2', b23s, 't2', 't3'),
                                ('sw4', b4s, 't3', 't4'))):
                            for j in tiles:
                                p = psw.tile([128, tw], F32, tag="p", bufs=p_bufs,
                                             name=f"p{i}_{j}_{s}")
                                nc.tensor.matmul(p[:], rw[wnm][:], st[j][tin][:],
                                                 start=True, stop=True)
                                st[j]['p'] = p
                            for j in tiles:
                                nc.vector.tensor_tensor(
                                    st[j]['p'][:], st[j]['p'][:], st[j]['u'][:], ADD)
                            for j in tiles:
                                tn = sb.tile([128, tw], F32R, tag="t", bufs=t_bufs,
                                             name=f"{tout}_{j}_{s}")
                                nc.scalar.activation(tn[:], st[j]['p'][:], TANH,
                                                     bias=bias, scale=1.0)
                                st[j][tout] = tn

                        # t-sums on gpsimd (SBUF-only engine)
                        for j in tiles:
                            s14 = sb.tile([128, tw], F32R, tag="ss", bufs=ss_bufs,
                                          name=f"s14_{j}_{s}")
                            nc.gpsimd.tensor_tensor(s14[:], st[j]['t1'][:],
                                                    st[j]['t4'][:], ADD)
                            s23 = sb.tile([128, tw], F32R, tag="ss", bufs=ss_bufs,
                                          name=f"s23_{j}_{s}")
                            nc.gpsimd.tensor_tensor(s23[:], st[j]['t2'][:],
                                                    st[j]['t3'][:], ADD)
                            st[j]['s14'], st[j]['s23'] = s14, s23

                        # u' accumulation chain + state update
                        for j in tiles:
                            up = psw.tile([128, tw], F32, tag="p", bufs=p_bufs,
                                          name=f"up_{j}_{s}")
                            nc.tensor.matmul(up[:], rw['uw6'][:], st[j]['s14'][:],
                                             start=True, stop=False)
                            nc.tensor.matmul(up[:], rw['uw3hi'][:], st[j]['s23'][:],
                                             start=False, stop=False)
                            nc.tensor.matmul(up[:], rw['uw3lo'][:], st[j]['s23'][:],
                                             start=False, stop=True)
                            st[j]['up'] = up
                        for j in tiles:
                            un = sb.tile([128, tw], F32R, tag="u", bufs=u_bufs,
                                         name=f"u_{j}_{s}")
                            nc.vector.tensor_tensor(un[:], st[j]['up'][:],
                                                    st[j]['u'][:], ADD)
                            st[j]['u'] = un

                    # out tile: G^T u (hi/lo), PSUM -> out_sb via ACT copy
                    for j in tiles:
                        go = psw.tile([2 * D_OUT, tw], F32, tag="p", bufs=p_bufs,
                                      name=f"go_{j}")
                        nc.tensor.matmul(go[:], rw['gwhi'][:], st[j]['u'][:],
                                         start=True, stop=False)
                        nc.tensor.matmul(go[:], rw['gwlo'][:], st[j]['u'][:],
                                         start=False, stop=True)
                        nc.scalar.activation(out_sb[:, tw * j:tw * (j + 1)],
                                             go[:], COPY, bias=0.0, scale=1.0)

                for g0 in range(0, n_tiles, ilv):
                    emit_group(list(range(g0, min(g0 + ilv, n_tiles))))

                nc.sync.dma_start(y_d[:], out_sb[:])

    nc.compile()
    return nc


_NC_CACHE = {}


def _get_nc(n_tiles, n_steps):
    key = (n_tiles, n_steps)
    if key not in _NC_CACHE:
        _NC_CACHE[key] = build_nc(n_tiles, n_steps)
    return _NC_CACHE[key]


def _build_in_maps(inputs):
    host, u0Ts, occ = _precompute(**inputs)
    shared = {k: np.ascontiguousarray(v.astype(np.float32)) for k, v in host.items()}
    in_maps = []
    for i in range(N_CORES):
        m = dict(shared)
        m['u0T'] = u0Ts[i]
        in_maps.append(m)
    return in_maps, occ


def _run(inputs, trace=False):
    n_tiles = B_FULL // N_CORES // (2 * TW)
    nc = _get_nc(n_tiles, NS)
    in_maps, occ = _build_in_maps(inputs)
    res = run_bass_kernel_spmd(nc, in_maps, core_ids=list(range(N_CORES)),
                               trace=trace)
    bc = B_FULL // N_CORES
    half = bc // 2
    out = np.empty((B_FULL, D_OUT), np.float32)
    for i in range(N_CORES):
        yT = res.results[i]['yT']
        out[i * bc:i * bc + half] = yT[0:D_OUT].T + occ
        out[i * bc + half:(i + 1) * bc] = yT[D_OUT:2 * D_OUT].T + occ
    return out, res


def kernel(**inputs):
    out, _ = _run(inputs, trace=False)
    return out
